# revision 1
# baseline (speedup 1.0000x reference)
"""Trainium2 Bass kernel: 5-head MHA + post-softmax A_ds weighting + fc
+ ELU adapter + residual + LayerNorm  (nn_MultiHeadAttention_89472758710361).

Sharding: data-parallel over batch — 16 batches -> 2 per core x 8 cores.
All inputs replicated except input_Q/K/V (batch-sharded). No collectives.

v2: fp16 datapath everywhere (fp16 matmuls run 1 cycle/row on the TRN2 PE
and keep tf32-class precision: 10-bit mantissa; halves DMA bytes and
enables 2x/4x DVE modes). Per-batch phase structure tuned so each phase
is bound by a single saturated engine:

  startup     DMA-paced: wq/xq -> wk/xk -> wv/xv -> A^T, per-chunk issue
  proj burst  PE-bound: all 5 heads' Q/K projections up front (their
              PSUM->SBUF copies stay OUT of the attention exp window),
              then V = Xv @ Wv
  attention   ACT-bound (exps run back to back); per head h:
                S^T[k,q] = KT.T @ QT     (PE, PSUM, 2-deep pipelined)
                E = exp(S^T/sqrt(dk))    (ACT, PSUM->SBUF fp16)
                exp-sums: kb0-2 on GPSIMD, kb3-7 on DVE (two separate
                partition_all_reduce calls so neither engine queue ever
                waits on the other); reciprocal+normalize deferred into
                the NEXT head's kb loop at staggered points
                EA = E * A^T[kblk]       (DVE fp16 2x)
                ctxT[dv,q] += V.T @ EA   (PE, PSUM accumulate)
  fc phase    PE-bound: per q-block PSUM = ctx@Wfc + hidT.T@[a2_w;a2_b],
              +residual (DVE), LN stats bn_stats/aggr, rstd via ACT Sqrt
              + DVE recip (no Ln/Exp act-table thrash; a live dummy exp
              hoists the table reload off the critical path), LN apply as
              one DVE tensor_scalar, fp16 DMA out (host upcasts);
              hidT = elu(W1'^T @ ctxT) computed directly transposed, per
              q-half; next batch's V+Q/K projection bursts fill the
              drain/LayerNorm tail; the last head finalizes in q-halves
              so fc matmuls on ctxT[:,4] start ~2us early
"""

import math

import numpy as np

import concourse.bacc as bacc
import concourse.bass as bass
import concourse.tile as tile
from concourse import bass_isa, mybir
from concourse.bass_utils import run_bass_kernel_spmd

F32 = mybir.dt.float32
F16 = mybir.dt.float16
AF = mybir.ActivationFunctionType
ALU = mybir.AluOpType

B, L, D = 16, 1024, 640
H, DK, DV = 5, 128, 128
NCORES = 8
BPC = B // NCORES  # batches per core
NDC = D // 128     # 5 d_model chunks
NQB = L // 128     # 8 q blocks
NKB = L // 128     # 8 k blocks
SCALE = 1.0 / math.sqrt(DK)
LN_EPS = 1e-5


def build_module() -> bass.Bass:
    nc = bacc.Bacc("TRN2", target_bir_lowering=False)

    # ---- DRAM I/O (per-core shard shapes) ----
    xqT_d = nc.dram_tensor("xqT", [BPC, NDC, 128, L], F16, kind="ExternalInput")[:]
    xkT_d = nc.dram_tensor("xkT", [BPC, NDC, 128, L], F16, kind="ExternalInput")[:]
    xvT_d = nc.dram_tensor("xvT", [BPC, NDC, 128, L], F16, kind="ExternalInput")[:]
    res_d = nc.dram_tensor("resid", [BPC, NQB, 128, D], F16, kind="ExternalInput")[:]
    at_d = nc.dram_tensor("at", [NKB, 128, L], F16, kind="ExternalInput")[:]
    wq_d = nc.dram_tensor("wq", [NDC, 128, D], F16, kind="ExternalInput")[:]
    wk_d = nc.dram_tensor("wk", [NDC, 128, D], F16, kind="ExternalInput")[:]
    wv_d = nc.dram_tensor("wv", [NDC, 128, D], F16, kind="ExternalInput")[:]
    wfc_d = nc.dram_tensor("wfc", [H, 128, D], F16, kind="ExternalInput")[:]
    w1p_d = nc.dram_tensor("w1p", [H, 128, 8], F16, kind="ExternalInput")[:]
    a1b_d = nc.dram_tensor("a1b", [8, 1], F32, kind="ExternalInput")[:]
    a2s_d = nc.dram_tensor("a2s", [9, D], F16, kind="ExternalInput")[:]
    out_d = nc.dram_tensor("out", [BPC, L, D], F16, kind="ExternalOutput")[:]
    scr_d = nc.dram_tensor("scr", [BPC, 1], F32, kind="Internal")[:]

    with tile.TileContext(nc) as tc:
        with (
            tc.tile_pool(name="consts", bufs=1) as cp,
            tc.tile_pool(name="xp", bufs=16) as xp,
            tc.tile_pool(name="big", bufs=1) as bp,
            tc.tile_pool(name="work", bufs=2) as kp,
            tc.tile_pool(name="ps", bufs=1, space="PSUM") as ps,
        ):
            # ---- weights + constants (loaded once) ----
            # DMA issue order tracks first use: wq/xq chunks feed the very
            # first matmul, then wk/xk, then wv/xv + A^T, then fc consts
            wqs, wks, wv, wfc = [], [], [], []
            xq_a, xk_a, xv_a = ([[] for _ in range(BPC)] for _ in range(3))
            for c in range(NDC):
                t = cp.tile([128, D], F16, name=f"wq{c}")
                nc.sync.dma_start(out=t, in_=wq_d[c])
                wqs.append(t)
                t = xp.tile([128, L], F16, tag="xch", name=f"xq0_{c}")
                nc.sync.dma_start(out=t, in_=xqT_d[0, c])
                xq_a[0].append(t)
            for c in range(NDC):
                t = cp.tile([128, D], F16, name=f"wk{c}")
                nc.sync.dma_start(out=t, in_=wk_d[c])
                wks.append(t)
                t = xp.tile([128, L], F16, tag="xch", name=f"xk0_{c}")
                nc.sync.dma_start(out=t, in_=xkT_d[0, c])
                xk_a[0].append(t)
            at_sb = cp.tile([128, NKB, L], F16, name="at_sb")
            for c in range(NDC):
                t = cp.tile([128, D], F16, name=f"wv{c}")
                nc.sync.dma_start(out=t, in_=wv_d[c])
                wv.append(t)
                t = xp.tile([128, L], F16, tag="xch", name=f"xv0_{c}")
                nc.sync.dma_start(out=t, in_=xvT_d[0, c])
                xv_a[0].append(t)
            # A^T split per k-block: head 0's E*A waits only on its chunk
            for kb in range(NKB):
                nc.sync.dma_start(out=at_sb[:, kb, :], in_=at_d[kb])
            for h in range(H):
                t = cp.tile([128, D], F16, name=f"wfc{h}")
                nc.sync.dma_start(out=t, in_=wfc_d[h])
                wfc.append(t)
            w1p_sb = cp.tile([128, H, 8], F16, name="w1p_sb")
            nc.sync.dma_start(out=w1p_sb, in_=w1p_d.rearrange("c p n -> p c n"))
            a1b_sb = cp.tile([8, 1], F32, name="a1b_sb")
            nc.sync.dma_start(out=a1b_sb, in_=a1b_d)
            a2s_sb = cp.tile([9, D], F16, name="a2s_sb")
            nc.sync.dma_start(out=a2s_sb, in_=a2s_d)
            eps_sb = cp.tile([128, 1], F32, name="eps_sb")
            nc.vector.memset(eps_sb, LN_EPS)
            # batch-1 inputs: prefetched now, land during batch-0 attention
            for b in range(1, BPC):
                for nm, dr, lst in (("xq", xqT_d, xq_a), ("xk", xkT_d, xk_a),
                                    ("xv", xvT_d, xv_a)):
                    for c in range(NDC):
                        t = xp.tile([128, L], F16, tag="xch", name=f"{nm}{b}_{c}")
                        nc.sync.dma_start(out=t, in_=dr[b, c])
                        lst[b].append(t)

            for b in range(BPC):
                xq, xk, xv = xq_a[b], xk_a[b], xv_a[b]

                def project_qk_all(xsq, xsk, bb):
                    """All 5 heads' Q and K [dk, L] projections into two
                    [128, H, L] tiles, interleaved per head and split into
                    512-halves rotating through PSUM A slots so the ACT
                    evacuation of one half overlaps matmuls of the next.
                    Keeping these copies out of the attention window leaves
                    ACT to the exp stream."""
                    dq = kp.tile([128, H, L], F16, tag="qta", name=f"qta{bb}")
                    dk_ = kp.tile([128, H, L], F16, tag="kta", name=f"kta{bb}")
                    if bb == 0:
                        # batch 0 is DMA-paced: emit all Q units before any
                        # K unit so the in-order PE queue never waits on
                        # the later-arriving xk chunks
                        order = [(dq, xsq, wqs, h) for h in range(H)] + \
                                [(dk_, xsk, wks, h) for h in range(H)]
                    else:
                        order = [(t, x, w, h) for h in range(H)
                                 for t, x, w in ((dq, xsq, wqs),
                                                 (dk_, xsk, wks))]
                    for dst, xs, ws, h in order:
                        if True:
                            for qs in (0, 512):
                                pp = ps.tile([128, 512], F32, tag="A", bufs=3,
                                             name=f"pp{bb}_{h}_{qs}")
                                for c in range(NDC):
                                    nc.tensor.matmul(
                                        pp,
                                        lhsT=ws[c][:, h * 128 : (h + 1) * 128],
                                        rhs=xs[c][:, qs : qs + 512],
                                        start=(c == 0),
                                        stop=(c == NDC - 1),
                                        skip_group_check=True,
                                    )
                                nc.scalar.copy(out=dst[:, h, qs : qs + 512],
                                               in_=pp)
                    return dq, dk_

                def project_v(xvb, bb, evac):
                    v_sbn = bp.tile([128, NKB, D], F16, tag="V",
                                    name=f"v_sb{bb}")
                    for lc in range(NKB):
                        # [128, 2, 512]: each 320-wide matmul output stays
                        # in one PSUM bank (matmul out cannot cross a bank)
                        vps = ps.tile([128, 2, 512], F32, tag="A", bufs=3,
                                      name=f"vps{bb}_{lc}")
                        for c in range(NDC):
                            for i in range(2):
                                nc.tensor.matmul(
                                    vps[:, i, 0:320],
                                    lhsT=xvb[c][:, lc * 128 : (lc + 1) * 128],
                                    rhs=wv[c][:, i * 320 : i * 320 + 320],
                                    start=(c == 0),
                                    stop=(c == NDC - 1),
                                    skip_group_check=True,
                                )
                        dst = v_sbn[:, lc, :].rearrange("p (a b) -> p a b", a=2)
                        if evac == "act":
                            nc.scalar.copy(out=dst, in_=vps[:, :, 0:320])
                        else:
                            nc.vector.tensor_copy(dst, vps[:, :, 0:320])
                    return v_sbn

                # ---- all projections (batch 0 only; later batches are
                # emitted as PE fillers in the previous batch's fc) ----
                if b == 0:
                    qta, kta = project_qk_all(xq, xk, 0)
                    v_sb = None  # emitted inside head 0, after st(0)/st(1)
                else:
                    qta, kta, v_sb = nb_qt, nb_kt, nb_v

                # ================= attention per head =================
                # Each head's finalize is spread into the NEXT head's kb
                # loop (reduce of the DVE-side accumulator at kb0, merge at
                # kb2, divide at the end) so it never sits in any engine
                # queue ahead of the exp/ea streams that feed the PE, and
                # the two partition reduces depend on only one engine each.
                ctxT = bp.tile([128, H, L], F16, tag="CTX", name=f"ctxT{b}")
                pend = None

                for h in range(H):
                    ctx_ps = ps.tile([128, L], F32, tag="ctx", bufs=1,
                                     name=f"ctx{b}_{h}")
                    acc = kp.tile([128, L], F16, tag="acc", bufs=2,
                                  name=f"acc{b}_{h}")

                    # software-pipelined: keep the PE stream 2 k-blocks
                    # ahead on the S^T matmuls
                    st_t = {}

                    def emit_st(kb, h=h):
                        st = ps.tile([128, L], F32, tag="A", bufs=3,
                                     name=f"st{b}_{h}_{kb}")
                        for qs in (0, 512):
                            nc.tensor.matmul(
                                st[:, qs : qs + 512],
                                lhsT=kta[:, h, kb * 128 : (kb + 1) * 128],
                                rhs=qta[:, h, qs : qs + 512],
                                start=True,
                                stop=True,
                                skip_group_check=True,
                            )
                        st_t[kb] = st

                    emit_st(0)
                    emit_st(1)
                    if v_sb is None:
                        # V projection emitted AFTER the first two S^T
                        # groups: the exp/ea streams start earlier and
                        # overlap the V matmuls
                        v_sb = project_v(xv, b, "dve")
                    # kb 0-3 partial exp-sums accumulate on the (otherwise
                    # idle) GPSIMD engine, kb 4-7 on DVE; one DVE add joins
                    acc2 = kp.tile([128, L], F16, tag="acc2", bufs=2,
                                   name=f"acc2{b}_{h}")
                    dn1 = kp.tile([128, L], F16, tag="dn1", bufs=2,
                                  name=f"dn1{b}_{h}")
                    dn2 = kp.tile([128, L], F16, tag="dn2", bufs=2,
                                  name=f"dn2{b}_{h}")
                    es_l = {}
                    for kb in range(NKB):
                        st = st_t.pop(kb)
                        es = kp.tile([128, L], F16, tag="es", bufs=7,
                                     name=f"es{b}_{h}_{kb}")
                        nc.scalar.activation(out=es, in_=st, func=AF.Exp,
                                             scale=SCALE)
                        es_l[kb] = es
                        if kb + 2 < NKB:
                            emit_st(kb + 2)
                        if kb == 1:
                            nc.gpsimd.tensor_add(acc, es_l.pop(0), es)
                        elif kb == 2:
                            nc.gpsimd.tensor_add(acc, acc, es)
                        elif kb == 3:
                            # Pool-side reduce: depends only on Pool's own
                            # adds, so it launches mid-head
                            nc.gpsimd.partition_all_reduce(
                                dn1, acc, 128, bass_isa.ReduceOp.add)
                        elif kb == 4:
                            nc.vector.tensor_add(acc2, es_l.pop(3), es)
                        elif kb >= 5:
                            nc.vector.tensor_add(acc2, acc2, es)
                        ea = kp.tile([128, L], F16, tag="ea", bufs=5,
                                     name=f"ea{b}_{h}_{kb}")
                        nc.vector.tensor_mul(ea, es, at_sb[:, kb, :])
                        for qs in (0, 512):
                            nc.tensor.matmul(
                                ctx_ps[:, qs : qs + 512],
                                lhsT=v_sb[:, kb, h * 128 : (h + 1) * 128],
                                rhs=ea[:, qs : qs + 512],
                                start=(kb == 0),
                                stop=(kb == NKB - 1),
                                skip_group_check=True,
                            )
                        if pend is not None:
                            phh, pctxu, pdn1, pdn2, pacc2 = pend
                            if kb == 0:
                                nc.gpsimd.partition_all_reduce(
                                    pdn2, pacc2, 128, bass_isa.ReduceOp.add)
                            elif kb == 2:
                                nc.vector.tensor_add(pdn1, pdn1, pdn2)
                            elif kb in (3, 5):
                                s = slice(0, 512) if kb == 3 else slice(512, L)
                                with nc.allow_low_precision(
                                    reason="fp16 softmax denominators: "
                                    "~1e-3 relative, inside the 2e-2 budget"
                                ):
                                    nc.vector.reciprocal(out=pdn2[:, s],
                                                         in_=pdn1[:, s])
                    if pend is not None:
                        phh, pctxu, pdn1, pdn2, pacc2 = pend
                        nc.vector.tensor_mul(ctxT[:, phh, :], pctxu, pdn2)
                        pend = None
                    # evacuate ctx PSUM (alternating engine)
                    if h == H - 1:
                        # last head: finalize immediately, in q-halves and
                        # straight from PSUM (no evacuation copy), so the
                        # fc matmuls on ctxT[:,4] start ~2us earlier
                        with nc.allow_low_precision(
                            reason="fp16 softmax denominators: ~1e-3 "
                            "relative, well inside the 2e-2 budget"
                        ):
                            for lo, hi in ((0, 512), (512, 1024)):
                                s = slice(lo, hi)
                                nc.gpsimd.partition_all_reduce(
                                    dn2[:, s], acc2[:, s], 128,
                                    bass_isa.ReduceOp.add)
                                nc.vector.tensor_add(dn1[:, s], dn1[:, s],
                                                     dn2[:, s])
                                nc.vector.reciprocal(out=dn2[:, s],
                                                     in_=dn1[:, s])
                                nc.vector.tensor_mul(ctxT[:, h, s],
                                                     ctx_ps[:, s], dn2[:, s])
                    else:
                        ctxu = kp.tile([128, L], F16, tag="ctxu", bufs=2,
                                       name=f"ctxu{b}_{h}")
                        nc.scalar.copy(out=ctxu, in_=ctx_ps)
                        pend = (h, ctxu, dn1, dn2, acc2)

                # ======== adapter hidden, directly transposed ========
                # hidT[0:8, q] = elu(W1'^T @ ctxT + a1b); row 8 = ones
                hidT = kp.tile([9, L], F16, tag="hidT", bufs=1, name=f"hidT{b}")
                nc.vector.memset(hidT, 1.0)  # row 8 stays 1.0 (a2 bias row)
                te = kp.tile([8, L], F16, tag="te", bufs=1, name=f"te{b}")
                hz = kp.tile([8, L], F16, tag="hz", bufs=1, name=f"hz{b}")
                hp_t = {}
                for i, qs in enumerate((0, 512)):
                    hp = ps.tile([8, 512], F32, tag="A" if qs == 0 else "ctx",
                                 bufs=3 if qs == 0 else 1,
                                 name=f"hp{b}_{qs}")
                    for h in range(4):
                        nc.tensor.matmul(
                            hp,
                            lhsT=w1p_sb[:, h, :],
                            rhs=ctxT[:, h, qs : qs + 512],
                            start=(h == 0),
                            stop=False,
                            skip_group_check=True,
                        )
                    hp_t[qs] = hp

                def hp_finish(qs):
                    nc.tensor.matmul(
                        hp_t[qs],
                        lhsT=w1p_sb[:, 4, :],
                        rhs=ctxT[:, 4, qs : qs + 512],
                        start=False,
                        stop=True,
                        skip_group_check=True,
                    )

                # ================= fc + residual + LayerNorm =========
                # emit 3 q-blocks of the h<4 partial sums first: they only
                # need ctxT[0:4] and keep the PE busy through the h=4
                # denominator tail and the ELU chain
                xps_t, xsb_l, last_stdv = {}, [], [None]
                mv_a = kp.tile([128, NQB, 2], F32, tag="stats", bufs=1,
                               name=f"mv{b}")

                def fc_partial(qb, tagn):
                    xps = ps.tile([128, 2, 512], F32, tag=tagn,
                                  bufs=3 if tagn == "A" else 1,
                                  name=f"xps{b}_{qb}")
                    for i in range(2):
                        n0 = i * 320
                        for h in range(4):
                            nc.tensor.matmul(
                                xps[:, i, 0:320],
                                lhsT=ctxT[:, h, qb * 128 : (qb + 1) * 128],
                                rhs=wfc[h][:, n0 : n0 + 320],
                                start=(h == 0),
                                stop=False,
                                skip_group_check=True,
                            )
                    xps_t[qb] = xps

                def fc_h4(qb):
                    xps = xps_t[qb]
                    for i in range(2):
                        n0 = i * 320
                        nc.tensor.matmul(
                            xps[:, i, 0:320],
                            lhsT=ctxT[:, 4, qb * 128 : (qb + 1) * 128],
                            rhs=wfc[4][:, n0 : n0 + 320],
                            start=False,
                            stop=False,
                            skip_group_check=True,
                        )

                def fc_a2s(qb):
                    xps = xps_t[qb]
                    for i in range(2):
                        n0 = i * 320
                        nc.tensor.matmul(
                            xps[:, i, 0:320],
                            lhsT=hidT[:, qb * 128 : (qb + 1) * 128],
                            rhs=a2s_sb[:, n0 : n0 + 320],
                            start=False,
                            stop=True,
                            skip_group_check=True,
                        )

                def fc_drain(qb):
                    xps = xps_t.pop(qb)
                    rt = rt_t[qb]
                    xsb = kp.tile([128, D], F16, tag="xsb", bufs=8,
                                  name=f"xsb{b}_{qb}")
                    nc.vector.tensor_add(
                        xsb.rearrange("p (a b) -> p a b", a=2),
                        xps[:, :, 0:320],
                        rt.rearrange("p (a b) -> p a b", a=2),
                    )
                    bst = kp.tile([128, 2, 6], F32, tag="bst", name=f"bst{b}_{qb}")
                    for i in range(2):
                        nc.vector.bn_stats(
                            out=bst[:, i, :], in_=xsb[:, i * 320 : (i + 1) * 320])
                    nc.vector.bn_aggr(out=mv_a[:, qb, :], in_=bst)
                    xsb_l.append(xsb)
                    # LayerNorm scale/shift per q-block: fully incremental,
                    # so the end-of-kernel tail chain is one block deep
                    mean = mv_a[:, qb : qb + 1, 0]
                    stdv = kp.tile([128, 1], F32, tag="rstd", name=f"sd{b}_{qb}")
                    nc.scalar.activation(out=stdv, in_=mv_a[:, qb : qb + 1, 1],
                                         func=AF.Sqrt, bias=eps_sb)
                    last_stdv[0] = stdv
                    rstd = kp.tile([128, 1], F32, tag="rstd2", name=f"rs{b}_{qb}")
                    nc.vector.reciprocal(out=rstd, in_=stdv)
                    nmr = kp.tile([128, 1], F32, tag="nmr", name=f"nm{b}_{qb}")
                    nc.vector.tensor_mul(nmr, mean, rstd)
                    nc.vector.tensor_scalar(
                        out=nmr, in0=nmr, scalar1=-1.0,
                        scalar2=None, op0=ALU.mult,
                    )
                    if b == BPC - 1:
                        # last batch: ACT is idle here (no projection
                        # bursts), so the apply moves off the DVE drain path
                        nc.scalar.activation(
                            out=xsb, in_=xsb, func=AF.Identity,
                            bias=nmr, scale=rstd,
                        )
                    else:
                        nc.vector.tensor_scalar(
                            out=xsb, in0=xsb, scalar1=rstd, scalar2=nmr,
                            op0=ALU.mult, op1=ALU.add,
                        )
                    nc.sync.dma_start(
                        out=out_d[b, qb * 128 : (qb + 1) * 128, :], in_=xsb)

                # prefetch all residual blocks now: the DMA engines are
                # idle in this phase and the drains never wait on them
                rt_t = {}
                for qb in range(NQB):
                    rt_t[qb] = kp.tile([128, D], F16, tag="resid", bufs=8,
                                       name=f"rt{b}_{qb}")
                    nc.sync.dma_start(out=rt_t[qb], in_=res_d[b, qb])
                fc_partial(0, "A")
                fc_partial(1, "A")
                # ELU of the adapter hidden per q-half (ACT exp + identity,
                # DVE combine): half 0 completes first so the a2s matmuls
                # for q-blocks 0-3 need not wait for the full hidT
                for qs in (0, 512):
                    s = slice(qs, qs + 512)
                    hp_finish(qs)
                    nc.scalar.activation(out=te[:, s], in_=hp_t[qs],
                                         func=AF.Exp, bias=a1b_sb)
                    nc.scalar.activation(out=hz[:, s], in_=hp_t[qs],
                                         func=AF.Identity, bias=a1b_sb)
                    nc.vector.tensor_scalar(
                        out=te[:, s], in0=te[:, s], scalar1=1.0, scalar2=0.0,
                        op0=ALU.subtract, op1=ALU.min,
                    )
                    nc.vector.tensor_scalar(
                        out=hz[:, s], in0=hz[:, s], scalar1=0.0, scalar2=None,
                        op0=ALU.max,
                    )
                    nc.vector.tensor_add(hidT[0:8, s], hz[:, s], te[:, s])
                for qb in range(2):
                    fc_h4(qb)
                for qb in range(2):
                    fc_a2s(qb)
                    fc_drain(qb)
                # descending order: the early q-blocks' drain chains overlap
                # the later q-blocks' matmuls, so the end-of-kernel tail is
                # one drain chain deep (qb0/qb1 were hoisted above)
                _FC_TAGS = {7: "A", 6: "A", 5: "ctx", 4: "A", 3: "ctx", 2: "A"}
                for qb in (7, 6, 5, 4, 3, 2):
                    fc_partial(qb, _FC_TAGS[qb])
                    fc_h4(qb)
                    fc_a2s(qb)
                    fc_drain(qb)
                # ---- next batch's projections: a ~30us burst of pure PE
                # work that fills the fc-drain and LayerNorm tail (V first:
                # the next attention needs v_sb within its first k-block)
                if b + 1 < BPC:
                    # dummy exp: hoists the sqrt->exp act-table reload into
                    # this idle window instead of before the next batch's
                    # first real exp (the 4-byte DMA keeps it from DCE)
                    du = kp.tile([1, 1], F32, tag="du", name=f"du{b}")
                    nc.scalar.activation(out=du, in_=last_stdv[0][0:1, :],
                                         func=AF.Exp)
                    nc.sync.dma_start(out=scr_d[b], in_=du)
                    nb_v = project_v(xv_a[b + 1], b + 1, "act")
                    nb_qt, nb_kt = project_qk_all(xq_a[b + 1], xk_a[b + 1],
                                                  b + 1)
    nc.compile()
    return nc


_NC_CACHE = None


def _get_module():
    global _NC_CACHE
    if _NC_CACHE is None:
        _NC_CACHE = build_module()
    return _NC_CACHE


def make_in_maps(inputs: dict) -> list[dict]:
    f = lambda x: np.ascontiguousarray(np.asarray(x, dtype=np.float32))
    iq, ik, iv = f(inputs["input_Q"]), f(inputs["input_K"]), f(inputs["input_V"])
    a_ds = f(inputs["A_ds"])
    wq, wk, wv, wfc = f(inputs["Wq"]), f(inputs["Wk"]), f(inputs["Wv"]), f(inputs["Wfc"])
    a1w, a1b = f(inputs["a1_w"]), f(inputs["a1_b"])
    a2w, a2b = f(inputs["a2_w"]), f(inputs["a2_b"])

    h16 = lambda x: np.ascontiguousarray(np.asarray(x, dtype=np.float16))
    shared = {
        "at": h16(np.ascontiguousarray(a_ds.T).reshape(NKB, 128, L)),
        "wq": h16(wq.reshape(NDC, 128, D)),
        "wk": h16(wk.reshape(NDC, 128, D)),
        "wv": h16(wv.reshape(NDC, 128, D)),
        "wfc": h16(wfc.reshape(H, 128, D)),
        "w1p": h16((wfc.astype(np.float64) @ a1w.astype(np.float64))
                   .astype(np.float32).reshape(H, 128, 8)),
        "a1b": np.ascontiguousarray(a1b.reshape(8, 1), dtype=np.float32),
        "a2s": h16(np.concatenate([a2w, a2b.reshape(1, D)], axis=0)),
    }

    in_maps = []
    for c in range(NCORES):
        sl = slice(c * BPC, (c + 1) * BPC)
        m = dict(shared)
        m["xqT"] = h16(np.ascontiguousarray(
            iq[sl].transpose(0, 2, 1)).reshape(BPC, NDC, 128, L))
        m["xkT"] = h16(np.ascontiguousarray(
            ik[sl].transpose(0, 2, 1)).reshape(BPC, NDC, 128, L))
        m["xvT"] = h16(np.ascontiguousarray(
            iv[sl].transpose(0, 2, 1)).reshape(BPC, NDC, 128, L))
        m["resid"] = h16(iq[sl].reshape(BPC, NQB, 128, D))
        in_maps.append(m)
    return in_maps


_JIT_CACHE = None


def _get_jitted():
    """Build the 8-core shard_map executable once per process.

    run_bass_kernel_spmd re-traces jax on every call (~250ms); caching the
    jitted callable makes repeat kernel() calls cheap."""
    global _JIT_CACHE
    if _JIT_CACHE is not None:
        return _JIT_CACHE
    import jax
    from jax.sharding import Mesh, PartitionSpec
    from jax.experimental.shard_map import shard_map
    from concourse import mybir
    from concourse.bass2jax import (
        _bass_exec_p, install_neuronx_cc_hook, partition_id_tensor)

    nc = _get_module()
    install_neuronx_cc_hook()
    pname = nc.partition_id_tensor.name if nc.partition_id_tensor else None
    in_names, out_names, out_avals, zero_shapes = [], [], [], []
    for alloc in nc.m.functions[0].allocations:
        if not isinstance(alloc, mybir.MemoryLocationSet):
            continue
        name = alloc.memorylocations[0].name
        if alloc.kind == "ExternalInput":
            if name != pname:
                in_names.append(name)
        elif alloc.kind == "ExternalOutput":
            shape = tuple(alloc.tensor_shape)
            dtype = mybir.dt.np(alloc.dtype)
            out_names.append(name)
            out_avals.append(jax.core.ShapedArray(shape, dtype))
            zero_shapes.append((shape, dtype))
    all_in = list(in_names) + list(out_names)
    if pname is not None:
        all_in.append(pname)

    def _body(*args):
        operands = list(args)
        if pname is not None:
            operands.append(partition_id_tensor())
        return tuple(_bass_exec_p.bind(
            *operands, out_avals=tuple(out_avals), in_names=tuple(all_in),
            out_names=tuple(out_names), lowering_input_output_aliases=(),
            sim_require_finite=True, sim_require_nnan=True, nc=nc))

    devices = jax.devices()[:NCORES]
    mesh = Mesh(np.asarray(devices), ("core",))
    n = len(in_names) + len(out_names)
    sharded = jax.jit(
        shard_map(_body, mesh=mesh, in_specs=(PartitionSpec("core"),) * n,
                  out_specs=(PartitionSpec("core"),) * len(out_names),
                  check_rep=False),
        keep_unused=True,
    )
    _JIT_CACHE = (sharded, in_names, zero_shapes)
    return _JIT_CACHE


def kernel(**inputs) -> np.ndarray:
    in_maps = make_in_maps(inputs)
    try:
        sharded, in_names, zero_shapes = _get_jitted()
        concat_in = [
            np.concatenate([np.asarray(in_maps[c][nm]) for c in range(NCORES)],
                           axis=0)
            for nm in in_names
        ]
        concat_zeros = [
            np.zeros((NCORES * s[0], *s[1:]), d) for s, d in zero_shapes
        ]
        outs = sharded(*concat_in, *concat_zeros)
        return np.asarray(outs[0]).astype(np.float32).reshape(B, L, D)
    except Exception:
        nc = _get_module()
        res = run_bass_kernel_spmd(nc, in_maps, core_ids=list(range(NCORES)))
        return np.concatenate([r["out"] for r in res.results],
                              axis=0).astype(np.float32)



# revision 27
# speedup vs baseline: 1.0548x; 1.0548x over previous
"""Trainium2 Bass kernel: 5-head MHA + post-softmax A_ds weighting + fc
+ ELU adapter + residual + LayerNorm  (nn_MultiHeadAttention_89472758710361).

Sharding: data-parallel over batch — 16 batches -> 2 per core x 8 cores.
All inputs replicated except input_Q/K/V (batch-sharded). No collectives.

v4: fp8(e4m3) DoubleRow matmuls for the d_model-contracting GEMMs
(Q/K/V projections, fc, adapter-hidden): two 128-deep contraction slabs
per instruction at 0.5 cycles/row = 4x fp16 PE throughput.  Attention
stays fp16 (S^T, ctx matmuls, exp/softmax).  Hardware constraints honored
throughout: GPSIMD touches SBUF only, and there is no tensor/tensor
divide anywhere (normalize = DVE reciprocal + Pool multiply, fp8 out).

Engine plan per attention head (the ACT exp stream, 8x1.04us, paces):
  ACT   8 exps + ctx PSUM evacuation (fp16 ctxu) at head end
  DVE   ea = es*A^T as 2-wide pair ops; denominator tree w4 (4-wide) +
        m2t (2-wide); reciprocal of the reduced denominator
  Pool  m = m2t[0]+m2t[1]; partition_all_reduce; normalize multiply
        (ctxu * recip -> fp8 ctxT6) — all SBUF-side, pipelined one head
        behind
  PE    S^T and ctx matmuls (~6.8us of 8.3)

Cross-batch software pipelining by EMISSION order (all queues are
in-order): after batch b's attention, only the next batch's projections
(+ their evacuations) and the adapter-hidden head-0..3 matmuls are
emitted; the whole fc/LayerNorm phase of batch b is chopped into small
closures that are injected one-per-kb-slot into batch b+1's attention
emission, so b+1's exp stream starts right after the projections while
b's drains/LayerNorm trail through b+1's engine slack.  The last batch
runs its fc inline.  LayerNorm rstd uses an integer-shift initial guess
plus two Newton steps on Pool (no ACT Sqrt: avoids act-table thrash
inside the exp stream).
"""

import math

import numpy as np

import concourse.bacc as bacc
import concourse.bass as bass
import concourse.tile as tile
from concourse import bass_isa, mybir
from concourse.bass_utils import run_bass_kernel_spmd

F32 = mybir.dt.float32
F16 = mybir.dt.float16
F8 = mybir.dt.float8e4
I32 = mybir.dt.int32
AF = mybir.ActivationFunctionType
ALU = mybir.AluOpType
PM = mybir.MatmulPerfMode

B, L, D = 16, 1024, 640
H, DK, DV = 5, 128, 128
NCORES = 8
BPC = B // NCORES  # batches per core
NDC = D // 128     # 5 real d_model chunks (padded to 6 on-chip)
NQB = L // 128     # 8 q blocks
NKB = L // 128     # 8 k blocks
SCALE = 1.0 / math.sqrt(DK)
LN_EPS = 1e-5


def build_module() -> bass.Bass:
    nc = bacc.Bacc("TRN2", target_bir_lowering=False)

    # ---- DRAM I/O (per-core shard shapes) ----
    xq_d = nc.dram_tensor("xq8", [BPC, 128, NDC + 1, L], F8, kind="ExternalInput")[:]
    xk_d = nc.dram_tensor("xk8", [BPC, 128, NDC + 1, L], F8, kind="ExternalInput")[:]
    xv_d = nc.dram_tensor("xv8", [BPC, 128, NDC + 1, L], F8, kind="ExternalInput")[:]
    res_d = nc.dram_tensor("resid", [BPC, NQB, 128, D], F16, kind="ExternalInput")[:]
    at_d = nc.dram_tensor("at", [128, NKB, L], F16, kind="ExternalInput")[:]
    wq_d = nc.dram_tensor("w8q", [128, NDC + 1, D], F8, kind="ExternalInput")[:]
    wk_d = nc.dram_tensor("w8k", [128, NDC + 1, D], F8, kind="ExternalInput")[:]
    wv_d = nc.dram_tensor("w8v", [128, NDC + 1, D], F8, kind="ExternalInput")[:]
    wfc_d = nc.dram_tensor("wfc8", [128, NDC + 1, D], F8, kind="ExternalInput")[:]
    w1p_d = nc.dram_tensor("w1p8", [128, NDC + 1, 8], F8, kind="ExternalInput")[:]
    a1b_d = nc.dram_tensor("a1b", [8, 1], F32, kind="ExternalInput")[:]
    a2s_d = nc.dram_tensor("a2s", [9, D], F16, kind="ExternalInput")[:]
    out_d = nc.dram_tensor("out", [BPC, L, D], F16, kind="ExternalOutput")[:]

    with tile.TileContext(nc) as tc:
        with (
            tc.tile_pool(name="consts", bufs=1) as cp,
            tc.tile_pool(name="xp", bufs=2) as xp,
            tc.tile_pool(name="big", bufs=1) as bp,
            tc.tile_pool(name="work", bufs=2) as kp,
            tc.tile_pool(name="ps", bufs=1, space="PSUM") as ps,
        ):
            # ---- weights + constants, one DMA per tensor (DMA issue costs
            # 650ns each on the SP queue; transfers parallelize inside) ----
            def load(name, pool, dram, width, tag=None):
                t = pool.tile([128, NDC + 1, width], F8, name=name, tag=tag)
                nc.sync.dma_start(out=t, in_=dram)
                return t

            wk = load("wk", cp, wk_d, D)
            xq_a, xk_a, xv_a = [], [], []
            xk_a.append(load("xk0", xp, xk_d[0], L, tag="xk"))
            wq = load("wq", cp, wq_d, D)
            xq_a.append(load("xq0", xp, xq_d[0], L, tag="xq"))
            wv = load("wv", cp, wv_d, D)
            xv_a.append(load("xv0", xp, xv_d[0], L, tag="xv"))
            at_sb = cp.tile([128, NKB, L], F16, name="at_sb")
            nc.sync.dma_start(out=at_sb, in_=at_d)
            wfc = load("wfc", cp, wfc_d, D)
            w1p = load("w1p", cp, w1p_d, 8)
            a1b_sb = cp.tile([8, 1], F32, name="a1b_sb")
            nc.sync.dma_start(out=a1b_sb, in_=a1b_d)
            a2s_sb = cp.tile([9, D], F16, name="a2s_sb")
            nc.sync.dma_start(out=a2s_sb, in_=a2s_d)
            # normalized fp8 context [dv, h, q]; head 5 stays zero as the
            # DoubleRow padding slab for the fc/hid matmuls
            ctxT6 = bp.tile([128, NDC + 1, L], F8, name="ctxT6")
            nc.gpsimd.memset(ctxT6[:, 5, :], 0.0)
            # batch-1 inputs: prefetched now, land during batch-0 attention
            for b in range(1, BPC):
                xk_a.append(load(f"xk{b}", xp, xk_d[b], L, tag="xk"))
                xq_a.append(load(f"xq{b}", xp, xq_d[b], L, tag="xq"))
                xv_a.append(load(f"xv{b}", xp, xv_d[b], L, tag="xv"))

            DRC = [(0, True, False), (2, False, False), (4, False, True)]

            def project_qk(bb):
                """All 5 heads' Q^T/K^T [dk, L] via fp8 DoubleRow chains
                (3 instrs: slabs (0,1),(2,3),(4,zero)).  K evacuates on ACT
                (it gates the next batch's S^T stream), Q on DVE."""
                dq = kp.tile([128, H, L], F16, tag="qta", bufs=1,
                             name=f"qta{bb}")
                dk_ = kp.tile([128, H, L], F16, tag="kta", bufs=1,
                              name=f"kta{bb}")
                for h in range(H):
                    for dst, xs, ws, eng in (
                        (dk_, xk_a[bb], wk, "act"),
                        (dq, xq_a[bb], wq, "dve"),
                    ):
                        pp = ps.tile([128, L], F32, tag="A", bufs=3,
                                     name=f"pp{bb}_{h}_{eng}")
                        for qs in (0, 512):
                            for c, st_, sp_ in DRC:
                                nc.tensor.matmul(
                                    pp[:, qs : qs + 512],
                                    lhsT=ws[:, c : c + 2,
                                            h * 128 : (h + 1) * 128],
                                    rhs=xs[:, c : c + 2, qs : qs + 512],
                                    start=st_,
                                    stop=sp_,
                                    perf_mode=PM.DoubleRow,
                                    skip_group_check=True,
                                )
                        if eng == "act":
                            nc.scalar.copy(out=dst[:, h, :], in_=pp)
                        else:
                            nc.vector.tensor_copy(dst[:, h, :], pp)
                return dq, dk_

            def project_v_block(bb, v_sbn, lc):
                """One V k-block; fp8 DR; evac on ACT so the vps PSUM slot
                recycles at exp pace (the evac interleaves the exp stream:
                V blocks are emitted just-in-time inside head 0)."""
                vps = ps.tile([128, 2, 512], F32, tag="A", bufs=3,
                              name=f"vps{bb}_{lc}")
                for i in range(2):
                    for c, st_, sp_ in DRC:
                        nc.tensor.matmul(
                            vps[:, i, 0:320],
                            lhsT=xv_a[bb][:, c : c + 2,
                                          lc * 128 : (lc + 1) * 128],
                            rhs=wv[:, c : c + 2, i * 320 : i * 320 + 320],
                            start=st_,
                            stop=sp_,
                            perf_mode=PM.DoubleRow,
                            skip_group_check=True,
                        )
                dst = v_sbn[:, lc, :].rearrange("p (a b) -> p a b", a=2)
                nc.scalar.copy(out=dst, in_=vps[:, :, 0:320])

            # fc/LN work of batch b-1, injected one closure per kb slot
            # into batch b's attention emission
            inject_q = []

            def attention(b, qta, kta, v_sb):
                for h in range(H):
                    es = kp.tile([128, NKB, L], F16, tag="es", bufs=2,
                                 name=f"es{b}_{h}")
                    es_r = es.rearrange("p (a b) q -> p a (b q)", b=2)
                    ctx_ps = ps.tile([128, L], F32, tag="ctx", bufs=1,
                                     name=f"ctx{b}_{h}")
                    st_t = {}

                    def emit_st(kb, h=h):
                        st = ps.tile([128, L], F32, tag="A", bufs=3,
                                     name=f"st{b}_{h}_{kb}")
                        for qs in (0, 512):
                            nc.tensor.matmul(
                                st[:, qs : qs + 512],
                                lhsT=kta[:, h, kb * 128 : (kb + 1) * 128],
                                rhs=qta[:, h, qs : qs + 512],
                                start=True,
                                stop=True,
                                skip_group_check=True,
                            )
                        st_t[kb] = st

                    emit_st(0)
                    emit_st(1)
                    if v_sb is None:
                        v_sb = bp.tile([128, NKB, D], F16, tag="V",
                                       name=f"v{b}")
                        v_fill = [0, 1]
                    else:
                        v_fill = []

                    dps = {}
                    for kb in range(NKB):
                        for lc in v_fill:
                            project_v_block(b, v_sb, lc)
                        v_fill = [kb + 2] if (v_fill and kb + 2 < NKB) else []
                        st = st_t.pop(kb)
                        nc.scalar.activation(out=es[:, kb, :], in_=st,
                                             func=AF.Exp, scale=SCALE)
                        if kb + 2 < NKB:
                            emit_st(kb + 2)
                        if kb % 2 == 1:
                            k0 = kb - 1
                            ea = kp.tile([128, 2, L], F16, tag="ea", bufs=2,
                                         name=f"ea{b}_{h}_{k0}")
                            nc.vector.tensor_mul(ea, es[:, k0 : k0 + 2, :],
                                                 at_sb[:, k0 : k0 + 2, :])
                            for j in (0, 1):
                                for qs in (0, 512):
                                    nc.tensor.matmul(
                                        ctx_ps[:, qs : qs + 512],
                                        lhsT=v_sb[:, k0 + j,
                                                  h * 128 : (h + 1) * 128],
                                        rhs=ea[:, j, qs : qs + 512],
                                        start=(k0 + j == 0),
                                        stop=(k0 + j == NKB - 1),
                                        skip_group_check=True,
                                    )
                        # last head: incremental denominator adds so the
                        # post-attention tail is short
                        if h == H - 1:
                            if kb % 2 == 1:
                                dp = kp.tile([128, L], F16, tag="dacc", bufs=2,
                                             name=f"dp{b}_{kb}")
                                nc.vector.tensor_add(dp, es[:, kb - 1, :],
                                                     es[:, kb, :])
                                dps[kb] = dp
                            elif kb == 4:
                                acc = kp.tile([128, L], F16, tag="dacc2",
                                              bufs=2, name=f"dac{b}_{kb}")
                                nc.vector.tensor_add(acc, dps[1], dps[3])
                                dps["a"] = acc
                            elif kb == 6:
                                acc = kp.tile([128, L], F16, tag="dacc2",
                                              bufs=2, name=f"dac{b}_{kb}")
                                nc.vector.tensor_add(acc, dps["a"], dps[5])
                                dps["a"] = acc
                        elif h >= 1 and inject_q:
                            inject_q.pop(0)()

                    # ---- denominator chain + ctx evac, head-end ----
                    with nc.allow_low_precision(
                        reason="fp16 softmax denominators + fp8 normalized "
                        "context: ~6e-3 rel, inside the 2e-2 budget"
                    ):
                        if h < H - 1:
                            w4 = kp.tile([128, 4, L], F16, tag="w4", bufs=1,
                                         name=f"w4{b}_{h}")
                            nc.vector.tensor_add(w4, es_r[:, :, 0:L],
                                                 es_r[:, :, L : 2 * L])
                            m2t = kp.tile([128, 2, L], F16, tag="m2t", bufs=1,
                                          name=f"m2t{b}_{h}")
                            nc.vector.tensor_add(m2t, w4[:, 0:2, :],
                                                 w4[:, 2:4, :])
                            ctxu = kp.tile([128, L], F16, tag="ctxu", bufs=2,
                                           name=f"ctxu{b}_{h}")
                            nc.scalar.copy(out=ctxu, in_=ctx_ps)
                            m = kp.tile([128, L], F16, tag="dm", bufs=1,
                                        name=f"m{b}_{h}")
                            nc.gpsimd.tensor_add(m, m2t[:, 0, :], m2t[:, 1, :])
                            dn = kp.tile([128, L], F16, tag="dn", bufs=2,
                                         name=f"dn{b}_{h}")
                            nc.gpsimd.partition_all_reduce(
                                dn, m, 128, bass_isa.ReduceOp.add)
                            rcp = kp.tile([128, L], F16, tag="rcp", bufs=1,
                                          name=f"rcp{b}_{h}")
                            nc.vector.reciprocal(out=rcp, in_=dn)
                            nc.gpsimd.tensor_mul(ctxT6[:, h, :], ctxu, rcp)
                        else:
                            # short tail: adds were interleaved above; the
                            # normalize multiply reads ctx PSUM on DVE
                            mh = kp.tile([128, L], F16, tag="dm4", bufs=1,
                                         name=f"m{b}_{h}")
                            nc.vector.tensor_add(mh, dps["a"], dps[7])
                            dnh = kp.tile([128, L], F16, tag="dn", bufs=2,
                                          name=f"dn{b}_{h}")
                            nc.gpsimd.partition_all_reduce(
                                dnh, mh, 128, bass_isa.ReduceOp.add)
                            rch = kp.tile([128, L], F16, tag="rcp", bufs=1,
                                          name=f"rcp{b}_{h}")
                            nc.vector.reciprocal(out=rch, in_=dnh)
                            nc.vector.tensor_mul(ctxT6[:, h, :], ctx_ps, rch)

            def fc_items(b, rt_all, hp_t, hidT, te, hz):
                """fc + LayerNorm of batch b as small closures."""
                items = []
                mv_a = kp.tile([128, NQB, 2], F32, tag="stats", bufs=1,
                               name=f"mv{b}")
                xps_t, xsb_t, ln_state = {}, {}, {}

                def hp_finish(qs):
                    def f():
                        nc.tensor.matmul(
                            hp_t[qs],
                            lhsT=w1p[:, 4, :],
                            rhs=ctxT6[:, 4, qs : qs + 512],
                            start=False,
                            stop=True,
                            skip_group_check=True,
                        )
                        s = slice(qs, qs + 512)
                        nc.scalar.activation(out=te[:, s], in_=hp_t[qs],
                                             func=AF.Exp, bias=a1b_sb)
                        nc.scalar.activation(out=hz[:, s], in_=hp_t[qs],
                                             func=AF.Identity, bias=a1b_sb)
                        nc.vector.tensor_scalar(
                            out=te[:, s], in0=te[:, s], scalar1=1.0,
                            scalar2=0.0, op0=ALU.subtract, op1=ALU.min)
                        nc.vector.tensor_scalar(
                            out=hz[:, s], in0=hz[:, s], scalar1=0.0,
                            scalar2=None, op0=ALU.max)
                        nc.gpsimd.tensor_add(hidT[0:8, s], hz[:, s], te[:, s])
                    return f

                def fc_wave_mm(qb):
                    def f():
                        xps = ps.tile([128, 2, 512], F32, tag="A", bufs=3,
                                      name=f"xps{b}_{qb}")
                        for i in range(2):
                            n0 = i * 320
                            for hh in (0, 2, 4):
                                nc.tensor.matmul(
                                    xps[:, i, 0:320],
                                    lhsT=ctxT6[:, hh : hh + 2,
                                               qb * 128 : (qb + 1) * 128],
                                    rhs=wfc[:, hh : hh + 2, n0 : n0 + 320],
                                    start=(hh == 0),
                                    stop=False,
                                    perf_mode=PM.DoubleRow,
                                    skip_group_check=True,
                                )
                            nc.tensor.matmul(
                                xps[:, i, 0:320],
                                lhsT=hidT[:, qb * 128 : (qb + 1) * 128],
                                rhs=a2s_sb[:, n0 : n0 + 320],
                                start=False,
                                stop=True,
                                skip_group_check=True,
                            )
                        xps_t[qb] = xps
                    return f

                def fc_drain(qb):
                    def f():
                        xps = xps_t.pop(qb)
                        xsb = kp.tile([128, D], F16, tag="xsb", bufs=8,
                                      name=f"xsb{b}_{qb}")
                        nc.vector.tensor_add(
                            xsb.rearrange("p (a b) -> p a b", a=2),
                            xps[:, :, 0:320],
                            rt_all[:, qb, :].rearrange("p (a b) -> p a b",
                                                       a=2),
                        )
                        bst = kp.tile([128, 2, 6], F32, tag="bst",
                                      name=f"bst{b}_{qb}")
                        for i in range(2):
                            nc.vector.bn_stats(
                                out=bst[:, i, :],
                                in_=xsb[:, i * 320 : (i + 1) * 320])
                        nc.vector.bn_aggr(out=mv_a[:, qb, :], in_=bst)
                        xsb_t[qb] = xsb
                    return f

                def ln_applies():
                    """Batched LN scale/shift.  rstd = 1/sqrt(var+eps) via
                    int-shift guess + two Newton steps, all on Pool (SBUF
                    only; no ACT Sqrt, so no act-table thrash)."""
                    ve = kp.tile([128, NQB], F32, tag="ve", name=f"ve{b}")
                    nc.vector.tensor_scalar(out=ve, in0=mv_a[:, :, 1],
                                            scalar1=LN_EPS, scalar2=None,
                                            op0=ALU.add)
                    ri = kp.tile([128, NQB], I32, tag="ri", bufs=1,
                                 name=f"ri{b}")
                    nc.vector.tensor_scalar(out=ri, in0=ve.bitcast(I32),
                                            scalar1=1, scalar2=None,
                                            op0=ALU.arith_shift_right)
                    nc.vector.tensor_scalar(out=ri, in0=ri, scalar1=-1,
                                            scalar2=0x5F3759DF, op0=ALU.mult,
                                            op1=ALU.add)
                    r = ri.bitcast(F32)
                    t = kp.tile([128, NQB], F32, tag="nt", name=f"nt{b}")
                    with nc.allow_low_precision(
                        reason="Newton rsqrt: 2 iterations give ~1e-5 rel"
                    ):
                        for _ in range(2):
                            nc.vector.tensor_mul(t, r, r)
                            nc.vector.tensor_mul(t, t, ve)
                            nc.vector.tensor_scalar(out=t, in0=t,
                                                    scalar1=-0.5, scalar2=1.5,
                                                    op0=ALU.mult, op1=ALU.add)
                            nc.vector.tensor_mul(r, r, t)
                        nm = kp.tile([128, NQB], F32, tag="nm", bufs=1,
                                     name=f"nm{b}")
                        nc.vector.tensor_mul(nm, mv_a[:, :, 0], r)
                        nc.vector.tensor_scalar(out=nm, in0=nm, scalar1=-1.0,
                                                scalar2=None, op0=ALU.mult)
                    ln_state["r"] = r
                    ln_state["nm"] = nm

                def apply_out(qb):
                    def f():
                        xsb = xsb_t.pop(qb)
                        r, nm = ln_state["r"], ln_state["nm"]
                        nc.vector.tensor_scalar(
                            out=xsb, in0=xsb, scalar1=r[:, qb : qb + 1],
                            scalar2=nm[:, qb : qb + 1],
                            op0=ALU.mult, op1=ALU.add,
                        )
                        nc.sync.dma_start(
                            out=out_d[b, qb * 128 : (qb + 1) * 128, :],
                            in_=xsb)
                    return f

                items.append(hp_finish(0))
                items.append(hp_finish(512))
                for qb in range(NQB):
                    items.append(fc_wave_mm(qb))
                    items.append(fc_drain(qb))
                items.append(ln_applies)
                applies = [apply_out(qb) for qb in range(NQB)]
                return items, applies

            nb = (None, None)
            apply_items = []
            for b in range(BPC):
                if b == 0:
                    qta, kta = project_qk(0)
                else:
                    qta, kta = nb
                # V is emitted inside head 0 for every batch: its PSUM tiles
                # then interleave with the st stream in the A ring instead of
                # stranding the first st behind 8 more allocations
                attention(b, qta, kta, None)

                # next batch's K/Q projections first: they gate its exp
                # stream (K/Q interleave per head so head 0 lands early)
                if b + 1 < BPC:
                    nb = project_qk(b + 1)

                # adapter hidden for heads 0-3 + residual prefetch
                hidT = kp.tile([9, L], F16, tag="hidT", bufs=1,
                               name=f"hidT{b}")
                nc.gpsimd.memset(hidT, 1.0)  # row 8 = a2 bias row
                te = kp.tile([8, L], F16, tag="te", bufs=1, name=f"te{b}")
                hz = kp.tile([8, L], F16, tag="hz", bufs=1, name=f"hz{b}")
                hp_t = {}
                for qs in (0, 512):
                    hp = ps.tile([8, 512], F32, tag="A" if qs == 0 else "ctx",
                                 bufs=3 if qs == 0 else 1, name=f"hp{b}_{qs}")
                    # plain fp8 (walrus rejects DoubleRow ldweights with an
                    # 8-wide stationary)
                    for hh in range(4):
                        nc.tensor.matmul(
                            hp,
                            lhsT=w1p[:, hh, :],
                            rhs=ctxT6[:, hh, qs : qs + 512],
                            start=(hh == 0),
                            stop=False,
                            skip_group_check=True,
                        )
                    hp_t[qs] = hp
                rt_all = kp.tile([128, NQB, D], F16, tag="resid", bufs=1,
                                 name=f"rt{b}")
                nc.sync.dma_start(out=rt_all,
                                  in_=res_d[b].rearrange("a p d -> p a d"))

                # previous batch's LN applies + out-DMAs go AFTER this
                # batch's residual DMA in the SP queue (a blocked out-DMA at
                # the SP head would starve the residual load for ~100us)
                for it in apply_items:
                    it()
                items, applies = fc_items(b, rt_all, hp_t, hidT, te, hz)
                INJECT = False
                if INJECT and b + 1 < BPC:
                    inject_q.extend(items)
                    apply_items = applies
                else:
                    for it in items:
                        it()
                    for it in applies:
                        it()
    nc.compile()
    return nc


_NC_CACHE = None


def _get_module():
    global _NC_CACHE
    if _NC_CACHE is None:
        _NC_CACHE = build_module()
    return _NC_CACHE


def make_in_maps(inputs: dict) -> list[dict]:
    f = lambda x: np.ascontiguousarray(np.asarray(x, dtype=np.float32))
    iq, ik, iv = f(inputs["input_Q"]), f(inputs["input_K"]), f(inputs["input_V"])
    a_ds = f(inputs["A_ds"])
    wq, wk, wv, wfc = f(inputs["Wq"]), f(inputs["Wk"]), f(inputs["Wv"]), f(inputs["Wfc"])
    a1w, a1b = f(inputs["a1_w"]), f(inputs["a1_b"])
    a2w, a2b = f(inputs["a2_w"]), f(inputs["a2_b"])

    h16 = lambda x: np.ascontiguousarray(np.asarray(x, dtype=np.float16))
    e4 = mybir.dt.np(F8)
    h8 = lambda x: np.ascontiguousarray(np.asarray(x, dtype=e4))

    def wpack(w, width=D):
        # [640, width] -> [128, 6, width] with (p, c, m) = w[128c+p, m];
        # chunk 5 is zero padding for the DoubleRow slab pairing
        t = np.zeros((128, NDC + 1, width), dtype=e4)
        t[:, :NDC, :] = h8(w.reshape(NDC, 128, width).transpose(1, 0, 2))
        return t

    w1p = (wfc.astype(np.float64) @ a1w.astype(np.float64)).astype(np.float32)
    shared = {
        "at": h16(np.ascontiguousarray(a_ds.T).reshape(NKB, 128, L)
                  .transpose(1, 0, 2)),
        "w8q": wpack(wq),
        "w8k": wpack(wk),
        "w8v": wpack(wv),
        "wfc8": wpack(wfc),
        "w1p8": wpack(w1p, width=8),
        "a1b": np.ascontiguousarray(a1b.reshape(8, 1), dtype=np.float32),
        "a2s": h16(np.concatenate([a2w, a2b.reshape(1, D)], axis=0)),
    }

    def xpack(x):
        # [BPC, L, D] -> [BPC, 128, 6, L] with (b, p, c, q) = x[b, q, 128c+p]
        t = np.zeros((BPC, 128, NDC + 1, L), dtype=e4)
        t[:, :, :NDC, :] = h8(x.transpose(0, 2, 1).reshape(BPC, NDC, 128, L)
                              .transpose(0, 2, 1, 3))
        return t

    in_maps = []
    for c in range(NCORES):
        sl = slice(c * BPC, (c + 1) * BPC)
        m = dict(shared)
        m["xq8"] = xpack(iq[sl])
        m["xk8"] = xpack(ik[sl])
        m["xv8"] = xpack(iv[sl])
        m["resid"] = h16(iq[sl].reshape(BPC, NQB, 128, D))
        in_maps.append(m)
    return in_maps


_JIT_CACHE = None


def _get_jitted():
    """Build the 8-core shard_map executable once per process.

    run_bass_kernel_spmd re-traces jax on every call (~250ms); caching the
    jitted callable makes repeat kernel() calls cheap."""
    global _JIT_CACHE
    if _JIT_CACHE is not None:
        return _JIT_CACHE
    import jax
    from jax.sharding import Mesh, PartitionSpec
    from jax.experimental.shard_map import shard_map
    from concourse import mybir
    from concourse.bass2jax import (
        _bass_exec_p, install_neuronx_cc_hook, partition_id_tensor)

    nc = _get_module()
    install_neuronx_cc_hook()
    pname = nc.partition_id_tensor.name if nc.partition_id_tensor else None
    in_names, out_names, out_avals, zero_shapes = [], [], [], []
    for alloc in nc.m.functions[0].allocations:
        if not isinstance(alloc, mybir.MemoryLocationSet):
            continue
        name = alloc.memorylocations[0].name
        if alloc.kind == "ExternalInput":
            if name != pname:
                in_names.append(name)
        elif alloc.kind == "ExternalOutput":
            shape = tuple(alloc.tensor_shape)
            dtype = mybir.dt.np(alloc.dtype)
            out_names.append(name)
            out_avals.append(jax.core.ShapedArray(shape, dtype))
            zero_shapes.append((shape, dtype))
    all_in = list(in_names) + list(out_names)
    if pname is not None:
        all_in.append(pname)

    def _body(*args):
        operands = list(args)
        if pname is not None:
            operands.append(partition_id_tensor())
        return tuple(_bass_exec_p.bind(
            *operands, out_avals=tuple(out_avals), in_names=tuple(all_in),
            out_names=tuple(out_names), lowering_input_output_aliases=(),
            sim_require_finite=True, sim_require_nnan=True, nc=nc))

    devices = jax.devices()[:NCORES]
    mesh = Mesh(np.asarray(devices), ("core",))
    n = len(in_names) + len(out_names)
    sharded = jax.jit(
        shard_map(_body, mesh=mesh, in_specs=(PartitionSpec("core"),) * n,
                  out_specs=(PartitionSpec("core"),) * len(out_names),
                  check_rep=False),
        keep_unused=True,
    )
    _JIT_CACHE = (sharded, in_names, zero_shapes)
    return _JIT_CACHE


def kernel(**inputs) -> np.ndarray:
    in_maps = make_in_maps(inputs)
    try:
        sharded, in_names, zero_shapes = _get_jitted()
        concat_in = [
            np.concatenate([np.asarray(in_maps[c][nm]) for c in range(NCORES)],
                           axis=0)
            for nm in in_names
        ]
        concat_zeros = [
            np.zeros((NCORES * s[0], *s[1:]), d) for s, d in zero_shapes
        ]
        outs = sharded(*concat_in, *concat_zeros)
        return np.asarray(outs[0]).astype(np.float32).reshape(B, L, D)
    except Exception:
        nc = _get_module()
        res = run_bass_kernel_spmd(nc, in_maps, core_ids=list(range(NCORES)))
        return np.concatenate([r["out"] for r in res.results],
                              axis=0).astype(np.float32)


# revision 35
# speedup vs baseline: 1.0678x; 1.0123x over previous
"""Trainium2 Bass kernel: 5-head MHA + post-softmax A_ds weighting + fc
+ ELU adapter + residual + LayerNorm  (nn_MultiHeadAttention_89472758710361).

Sharding: data-parallel over batch — 16 batches -> 2 per core x 8 cores.
All inputs replicated except input_Q/K/V (batch-sharded). No collectives.

v4: fp8(e4m3) DoubleRow matmuls for the d_model-contracting GEMMs
(Q/K/V projections, fc, adapter-hidden): two 128-deep contraction slabs
per instruction at 0.5 cycles/row = 4x fp16 PE throughput.  Attention
stays fp16 (S^T, ctx matmuls, exp/softmax).  Hardware constraints honored
throughout: GPSIMD touches SBUF only, and there is no tensor/tensor
divide anywhere (normalize = DVE reciprocal + Pool multiply, fp8 out).

Engine plan per attention head (the ACT exp stream, 8x1.04us, paces):
  ACT   8 exps + ctx PSUM evacuation (fp16 ctxu) at head end
  DVE   ea = es*A^T as 2-wide pair ops; denominator tree w4 (4-wide) +
        m2t (2-wide); reciprocal of the reduced denominator
  Pool  m = m2t[0]+m2t[1]; partition_all_reduce; normalize multiply
        (ctxu * recip -> fp8 ctxT6) — all SBUF-side, pipelined one head
        behind
  PE    S^T and ctx matmuls (~6.8us of 8.3)

Cross-batch software pipelining by EMISSION order (all queues are
in-order): after batch b's attention, only the next batch's projections
(+ their evacuations) and the adapter-hidden head-0..3 matmuls are
emitted; the whole fc/LayerNorm phase of batch b is chopped into small
closures that are injected one-per-kb-slot into batch b+1's attention
emission, so b+1's exp stream starts right after the projections while
b's drains/LayerNorm trail through b+1's engine slack.  The last batch
runs its fc inline.  LayerNorm rstd uses an integer-shift initial guess
plus two Newton steps on Pool (no ACT Sqrt: avoids act-table thrash
inside the exp stream).
"""

import math

import numpy as np

import concourse.bacc as bacc
import concourse.bass as bass
import concourse.tile as tile
from concourse import bass_isa, mybir
from concourse.bass_utils import run_bass_kernel_spmd

F32 = mybir.dt.float32
F16 = mybir.dt.float16
F8 = mybir.dt.float8e4
I32 = mybir.dt.int32
AF = mybir.ActivationFunctionType
ALU = mybir.AluOpType
PM = mybir.MatmulPerfMode

B, L, D = 16, 1024, 640
H, DK, DV = 5, 128, 128
NCORES = 8
BPC = B // NCORES  # batches per core
NDC = D // 128     # 5 real d_model chunks (padded to 6 on-chip)
NQB = L // 128     # 8 q blocks
NKB = L // 128     # 8 k blocks
SCALE = 1.0 / math.sqrt(DK)
LN_EPS = 1e-5


def build_module() -> bass.Bass:
    nc = bacc.Bacc("TRN2", target_bir_lowering=False)

    # ---- DRAM I/O (per-core shard shapes) ----
    xq_d = nc.dram_tensor("xq8", [BPC, 128, NDC + 1, L], F8, kind="ExternalInput")[:]
    xk_d = nc.dram_tensor("xk8", [BPC, 128, NDC + 1, L], F8, kind="ExternalInput")[:]
    xv_d = nc.dram_tensor("xv8", [BPC, 128, NDC + 1, L], F8, kind="ExternalInput")[:]
    res_d = nc.dram_tensor("resid", [BPC, NQB, 128, D], F16, kind="ExternalInput")[:]
    at_d = nc.dram_tensor("at", [128, NKB, L], F16, kind="ExternalInput")[:]
    wq_d = nc.dram_tensor("w8q", [128, NDC + 1, D], F8, kind="ExternalInput")[:]
    wk_d = nc.dram_tensor("w8k", [128, NDC + 1, D], F8, kind="ExternalInput")[:]
    wv_d = nc.dram_tensor("w8v", [128, NDC + 1, D], F8, kind="ExternalInput")[:]
    wfc_d = nc.dram_tensor("wfc8", [128, NDC + 1, D], F8, kind="ExternalInput")[:]
    w1p_d = nc.dram_tensor("w1p8", [128, NDC + 1, 8], F8, kind="ExternalInput")[:]
    a1b_d = nc.dram_tensor("a1b", [8, 1], F32, kind="ExternalInput")[:]
    a2s_d = nc.dram_tensor("a2s", [9, D], F16, kind="ExternalInput")[:]
    out_d = nc.dram_tensor("out", [BPC, L, D], F16, kind="ExternalOutput")[:]

    with tile.TileContext(nc) as tc:
        with (
            tc.tile_pool(name="consts", bufs=1) as cp,
            tc.tile_pool(name="xp", bufs=2) as xp,
            tc.tile_pool(name="big", bufs=1) as bp,
            tc.tile_pool(name="work", bufs=2) as kp,
            tc.tile_pool(name="ps", bufs=1, space="PSUM") as ps,
        ):
            # ---- weights + constants, one DMA per tensor (DMA issue costs
            # 650ns each on the SP queue; transfers parallelize inside) ----
            def load(name, pool, dram, width, tag=None):
                t = pool.tile([128, NDC + 1, width], F8, name=name, tag=tag)
                nc.sync.dma_start(out=t, in_=dram)
                return t

            wk = load("wk", cp, wk_d, D)
            xq_a, xk_a, xv_a = [], [], []
            xk_a.append(load("xk0", xp, xk_d[0], L, tag="xk"))
            wq = load("wq", cp, wq_d, D)
            xq_a.append(load("xq0", xp, xq_d[0], L, tag="xq"))
            wv = load("wv", cp, wv_d, D)
            xv_a.append(load("xv0", xp, xv_d[0], L, tag="xv"))
            at_sb = cp.tile([128, NKB, L], F16, name="at_sb")
            nc.sync.dma_start(out=at_sb, in_=at_d)
            wfc = load("wfc", cp, wfc_d, D)
            w1p = load("w1p", cp, w1p_d, 8)
            a1b_sb = cp.tile([8, 1], F32, name="a1b_sb")
            nc.sync.dma_start(out=a1b_sb, in_=a1b_d)
            a2s_sb = cp.tile([9, D], F16, name="a2s_sb")
            nc.sync.dma_start(out=a2s_sb, in_=a2s_d)
            # normalized fp8 context [dv, h, q]; head 5 stays zero as the
            # DoubleRow padding slab for the fc/hid matmuls
            ctxT6 = bp.tile([128, NDC + 1, L], F8, name="ctxT6")
            nc.gpsimd.memset(ctxT6[:, 5, :], 0.0)
            # batch-1 inputs: prefetched now, land during batch-0 attention
            for b in range(1, BPC):
                xk_a.append(load(f"xk{b}", xp, xk_d[b], L, tag="xk"))
                xq_a.append(load(f"xq{b}", xp, xq_d[b], L, tag="xq"))
                xv_a.append(load(f"xv{b}", xp, xv_d[b], L, tag="xv"))

            DRC = [(0, True, False), (2, False, False), (4, False, True)]

            def project_qk(bb):
                """All 5 heads' Q^T/K^T [dk, L] via fp8 DoubleRow chains
                (3 instrs: slabs (0,1),(2,3),(4,zero)).  K evacuates on ACT
                (it gates the next batch's S^T stream), Q on DVE."""
                dq = kp.tile([128, H, L], F16, tag="qta", bufs=1,
                             name=f"qta{bb}")
                dk_ = kp.tile([128, H, L], F16, tag="kta", bufs=1,
                              name=f"kta{bb}")
                q_eng = "dve" if bb == 0 else "act"
                for h in range(H):
                    for dst, xs, ws, eng in (
                        (dk_, xk_a[bb], wk, "act"),
                        (dq, xq_a[bb], wq, q_eng),
                    ):
                        pp = ps.tile([128, L], F32, tag="A", bufs=3,
                                     name=f"pp{bb}_{h}_{eng}")
                        for qs in (0, 512):
                            for c, st_, sp_ in DRC:
                                nc.tensor.matmul(
                                    pp[:, qs : qs + 512],
                                    lhsT=ws[:, c : c + 2,
                                            h * 128 : (h + 1) * 128],
                                    rhs=xs[:, c : c + 2, qs : qs + 512],
                                    start=st_,
                                    stop=sp_,
                                    perf_mode=PM.DoubleRow,
                                    skip_group_check=True,
                                )
                        if eng == "act":
                            nc.scalar.copy(out=dst[:, h, :], in_=pp)
                        else:
                            nc.vector.tensor_copy(dst[:, h, :], pp)
                return dq, dk_

            def project_v_block(bb, v_sbn, lc):
                """One V k-block; fp8 DR; evac on ACT so the vps PSUM slot
                recycles at exp pace (the evac interleaves the exp stream:
                V blocks are emitted just-in-time inside head 0)."""
                vps = ps.tile([128, 2, 512], F32, tag="A", bufs=3,
                              name=f"vps{bb}_{lc}")
                for i in range(2):
                    for c, st_, sp_ in DRC:
                        nc.tensor.matmul(
                            vps[:, i, 0:320],
                            lhsT=xv_a[bb][:, c : c + 2,
                                          lc * 128 : (lc + 1) * 128],
                            rhs=wv[:, c : c + 2, i * 320 : i * 320 + 320],
                            start=st_,
                            stop=sp_,
                            perf_mode=PM.DoubleRow,
                            skip_group_check=True,
                        )
                dst = v_sbn[:, lc, :].rearrange("p (a b) -> p a b", a=2)
                nc.scalar.copy(out=dst, in_=vps[:, :, 0:320])

            # fc/LN work of batch b-1, injected one closure per kb slot
            # into batch b's attention emission
            inject_q = []

            def attention(b, qta, kta, v_sb):
                for h in range(H):
                    es = kp.tile([128, NKB, L], F16, tag="es", bufs=2,
                                 name=f"es{b}_{h}")
                    es_r = es.rearrange("p (a b) q -> p a (b q)", b=2)
                    ctx_ps = ps.tile([128, L], F32, tag="ctx", bufs=1,
                                     name=f"ctx{b}_{h}")
                    st_t = {}

                    def emit_st(kb, h=h):
                        st = ps.tile([128, L], F32, tag="A", bufs=3,
                                     name=f"st{b}_{h}_{kb}")
                        for qs in (0, 512):
                            nc.tensor.matmul(
                                st[:, qs : qs + 512],
                                lhsT=kta[:, h, kb * 128 : (kb + 1) * 128],
                                rhs=qta[:, h, qs : qs + 512],
                                start=True,
                                stop=True,
                                skip_group_check=True,
                            )
                        st_t[kb] = st

                    emit_st(0)
                    emit_st(1)
                    if v_sb is None:
                        v_sb = bp.tile([128, NKB, D], F16, tag="V",
                                       name=f"v{b}")
                        v_fill = [0, 1]
                    else:
                        v_fill = []

                    dps = {}
                    for kb in range(NKB):
                        for lc in v_fill:
                            project_v_block(b, v_sb, lc)
                        v_fill = [kb + 2] if (v_fill and kb + 2 < NKB) else []
                        st = st_t.pop(kb)
                        nc.scalar.activation(out=es[:, kb, :], in_=st,
                                             func=AF.Exp, scale=SCALE)
                        if kb + 2 < NKB:
                            emit_st(kb + 2)
                        if kb % 2 == 1:
                            k0 = kb - 1
                            ea = kp.tile([128, 2, L], F16, tag="ea", bufs=2,
                                         name=f"ea{b}_{h}_{k0}")
                            nc.vector.tensor_mul(ea, es[:, k0 : k0 + 2, :],
                                                 at_sb[:, k0 : k0 + 2, :])
                            for j in (0, 1):
                                for qs in (0, 512):
                                    nc.tensor.matmul(
                                        ctx_ps[:, qs : qs + 512],
                                        lhsT=v_sb[:, k0 + j,
                                                  h * 128 : (h + 1) * 128],
                                        rhs=ea[:, j, qs : qs + 512],
                                        start=(k0 + j == 0),
                                        stop=(k0 + j == NKB - 1),
                                        skip_group_check=True,
                                    )
                        # last head: incremental denominator adds so the
                        # post-attention tail is short
                        if h == H - 1:
                            if kb % 2 == 1:
                                dp = kp.tile([128, L], F16, tag="dacc", bufs=2,
                                             name=f"dp{b}_{kb}")
                                nc.vector.tensor_add(dp, es[:, kb - 1, :],
                                                     es[:, kb, :])
                                dps[kb] = dp
                            elif kb == 4:
                                acc = kp.tile([128, L], F16, tag="dacc2",
                                              bufs=2, name=f"dac{b}_{kb}")
                                nc.vector.tensor_add(acc, dps[1], dps[3])
                                dps["a"] = acc
                            elif kb == 6:
                                acc = kp.tile([128, L], F16, tag="dacc2",
                                              bufs=2, name=f"dac{b}_{kb}")
                                nc.vector.tensor_add(acc, dps["a"], dps[5])
                                dps["a"] = acc
                        elif h >= 1 and inject_q:
                            inject_q.pop(0)()

                    # ---- denominator chain + ctx evac, head-end ----
                    with nc.allow_low_precision(
                        reason="fp16 softmax denominators + fp8 normalized "
                        "context: ~6e-3 rel, inside the 2e-2 budget"
                    ):
                        if h < H - 1:
                            w4 = kp.tile([128, 4, L], F16, tag="w4", bufs=1,
                                         name=f"w4{b}_{h}")
                            nc.vector.tensor_add(w4, es_r[:, :, 0:L],
                                                 es_r[:, :, L : 2 * L])
                            m2t = kp.tile([128, 2, L], F16, tag="m2t", bufs=1,
                                          name=f"m2t{b}_{h}")
                            nc.vector.tensor_add(m2t, w4[:, 0:2, :],
                                                 w4[:, 2:4, :])
                            ctxu = kp.tile([128, L], F16, tag="ctxu", bufs=2,
                                           name=f"ctxu{b}_{h}")
                            nc.scalar.copy(out=ctxu, in_=ctx_ps)
                            m = kp.tile([128, L], F16, tag="dm", bufs=1,
                                        name=f"m{b}_{h}")
                            nc.gpsimd.tensor_add(m, m2t[:, 0, :], m2t[:, 1, :])
                            dn = kp.tile([128, L], F16, tag="dn", bufs=2,
                                         name=f"dn{b}_{h}")
                            nc.gpsimd.partition_all_reduce(
                                dn, m, 128, bass_isa.ReduceOp.add)
                            rcp = kp.tile([128, L], F16, tag="rcp", bufs=1,
                                          name=f"rcp{b}_{h}")
                            nc.vector.reciprocal(out=rcp, in_=dn)
                            nc.gpsimd.tensor_mul(ctxT6[:, h, :], ctxu, rcp)
                        else:
                            # short tail: adds were interleaved above; the
                            # normalize multiply reads ctx PSUM on DVE
                            mh = kp.tile([128, L], F16, tag="dm4", bufs=1,
                                         name=f"m{b}_{h}")
                            nc.vector.tensor_add(mh, dps["a"], dps[7])
                            dnh = kp.tile([128, L], F16, tag="dn", bufs=2,
                                          name=f"dn{b}_{h}")
                            nc.gpsimd.partition_all_reduce(
                                dnh, mh, 128, bass_isa.ReduceOp.add)
                            rch = kp.tile([128, L], F16, tag="rcp", bufs=1,
                                          name=f"rcp{b}_{h}")
                            nc.vector.reciprocal(out=rch, in_=dnh)
                            nc.vector.tensor_mul(ctxT6[:, h, :], ctx_ps, rch)

            def fc_items(b, rt_all, hp_t, hidT, te, hz):
                """fc + LayerNorm of batch b as small closures."""
                items = []
                mv_a = kp.tile([128, NQB, 2], F32, tag="stats", bufs=1,
                               name=f"mv{b}")
                xps_t, xsb_t, ln_state = {}, {}, {}

                def hp_finish(qs):
                    def f():
                        nc.tensor.matmul(
                            hp_t[qs],
                            lhsT=w1p[:, 4, :],
                            rhs=ctxT6[:, 4, qs : qs + 512],
                            start=False,
                            stop=True,
                            skip_group_check=True,
                        )
                        s = slice(qs, qs + 512)
                        nc.scalar.activation(out=te[:, s], in_=hp_t[qs],
                                             func=AF.Exp, bias=a1b_sb)
                        nc.scalar.activation(out=hz[:, s], in_=hp_t[qs],
                                             func=AF.Identity, bias=a1b_sb)
                        nc.vector.tensor_scalar(
                            out=te[:, s], in0=te[:, s], scalar1=1.0,
                            scalar2=0.0, op0=ALU.subtract, op1=ALU.min)
                        nc.vector.tensor_scalar(
                            out=hz[:, s], in0=hz[:, s], scalar1=0.0,
                            scalar2=None, op0=ALU.max)
                        nc.gpsimd.tensor_add(hidT[0:8, s], hz[:, s], te[:, s])
                    return f

                def fc_wave_mm(qb):
                    def f():
                        xps = ps.tile([128, 2, 512], F32, tag="A", bufs=3,
                                      name=f"xps{b}_{qb}")
                        for i in range(2):
                            n0 = i * 320
                            for hh in (0, 2, 4):
                                nc.tensor.matmul(
                                    xps[:, i, 0:320],
                                    lhsT=ctxT6[:, hh : hh + 2,
                                               qb * 128 : (qb + 1) * 128],
                                    rhs=wfc[:, hh : hh + 2, n0 : n0 + 320],
                                    start=(hh == 0),
                                    stop=False,
                                    perf_mode=PM.DoubleRow,
                                    skip_group_check=True,
                                )
                            nc.tensor.matmul(
                                xps[:, i, 0:320],
                                lhsT=hidT[:, qb * 128 : (qb + 1) * 128],
                                rhs=a2s_sb[:, n0 : n0 + 320],
                                start=False,
                                stop=True,
                                skip_group_check=True,
                            )
                        xps_t[qb] = xps
                    return f

                def fc_drain(qb):
                    def f():
                        xps = xps_t.pop(qb)
                        xsb = kp.tile([128, D], F16, tag="xsb", bufs=8,
                                      name=f"xsb{b}_{qb}")
                        nc.vector.tensor_add(
                            xsb.rearrange("p (a b) -> p a b", a=2),
                            xps[:, :, 0:320],
                            rt_all[:, qb, :].rearrange("p (a b) -> p a b",
                                                       a=2),
                        )
                        bst = kp.tile([128, 2, 6], F32, tag="bst",
                                      name=f"bst{b}_{qb}")
                        for i in range(2):
                            nc.vector.bn_stats(
                                out=bst[:, i, :],
                                in_=xsb[:, i * 320 : (i + 1) * 320])
                        nc.vector.bn_aggr(out=mv_a[:, qb, :], in_=bst)
                        xsb_t[qb] = xsb
                    return f

                def ln_applies():
                    """Batched LN scale/shift.  rstd = 1/sqrt(var+eps) via
                    int-shift guess + two Newton steps, all on Pool (SBUF
                    only; no ACT Sqrt, so no act-table thrash)."""
                    ve = kp.tile([128, NQB], F32, tag="ve", name=f"ve{b}")
                    nc.vector.tensor_scalar(out=ve, in0=mv_a[:, :, 1],
                                            scalar1=LN_EPS, scalar2=None,
                                            op0=ALU.add)
                    ri = kp.tile([128, NQB], I32, tag="ri", bufs=1,
                                 name=f"ri{b}")
                    nc.vector.tensor_scalar(out=ri, in0=ve.bitcast(I32),
                                            scalar1=1, scalar2=None,
                                            op0=ALU.arith_shift_right)
                    nc.vector.tensor_scalar(out=ri, in0=ri, scalar1=-1,
                                            scalar2=0x5F3759DF, op0=ALU.mult,
                                            op1=ALU.add)
                    r = ri.bitcast(F32)
                    t = kp.tile([128, NQB], F32, tag="nt", name=f"nt{b}")
                    with nc.allow_low_precision(
                        reason="Newton rsqrt: 2 iterations give ~1e-5 rel"
                    ):
                        for _ in range(2):
                            nc.vector.tensor_mul(t, r, r)
                            nc.vector.tensor_mul(t, t, ve)
                            nc.vector.tensor_scalar(out=t, in0=t,
                                                    scalar1=-0.5, scalar2=1.5,
                                                    op0=ALU.mult, op1=ALU.add)
                            nc.vector.tensor_mul(r, r, t)
                        nm = kp.tile([128, NQB], F32, tag="nm", bufs=1,
                                     name=f"nm{b}")
                        nc.vector.tensor_mul(nm, mv_a[:, :, 0], r)
                        nc.vector.tensor_scalar(out=nm, in0=nm, scalar1=-1.0,
                                                scalar2=None, op0=ALU.mult)
                    ln_state["r"] = r
                    ln_state["nm"] = nm

                def apply_out(qb):
                    def f():
                        xsb = xsb_t.pop(qb)
                        r, nm = ln_state["r"], ln_state["nm"]
                        nc.vector.tensor_scalar(
                            out=xsb, in0=xsb, scalar1=r[:, qb : qb + 1],
                            scalar2=nm[:, qb : qb + 1],
                            op0=ALU.mult, op1=ALU.add,
                        )
                        nc.sync.dma_start(
                            out=out_d[b, qb * 128 : (qb + 1) * 128, :],
                            in_=xsb)
                    return f

                items.append(hp_finish(0))
                items.append(hp_finish(512))
                for qb in range(NQB):
                    items.append(fc_wave_mm(qb))
                    items.append(fc_drain(qb))
                items.append(ln_applies)
                applies = [apply_out(qb) for qb in range(NQB)]
                return items, applies

            nb = (None, None)
            apply_items = []
            for b in range(BPC):
                if b == 0:
                    qta, kta = project_qk(0)
                else:
                    qta, kta = nb
                # V is emitted inside head 0 for every batch: its PSUM tiles
                # then interleave with the st stream in the A ring instead of
                # stranding the first st behind 8 more allocations
                attention(b, qta, kta, None)

                # next batch's K/Q projections first: they gate its exp
                # stream (K/Q interleave per head so head 0 lands early)
                if b + 1 < BPC:
                    nb = project_qk(b + 1)

                # adapter hidden for heads 0-3 + residual prefetch
                hidT = kp.tile([9, L], F16, tag="hidT", bufs=1,
                               name=f"hidT{b}")
                nc.gpsimd.memset(hidT, 1.0)  # row 8 = a2 bias row
                te = kp.tile([8, L], F16, tag="te", bufs=1, name=f"te{b}")
                hz = kp.tile([8, L], F16, tag="hz", bufs=1, name=f"hz{b}")
                hp_t = {}
                for qs in (0, 512):
                    hp = ps.tile([8, 512], F32, tag="A" if qs == 0 else "ctx",
                                 bufs=3 if qs == 0 else 1, name=f"hp{b}_{qs}")
                    # plain fp8 (walrus rejects DoubleRow ldweights with an
                    # 8-wide stationary)
                    for hh in range(4):
                        nc.tensor.matmul(
                            hp,
                            lhsT=w1p[:, hh, :],
                            rhs=ctxT6[:, hh, qs : qs + 512],
                            start=(hh == 0),
                            stop=False,
                            skip_group_check=True,
                        )
                    hp_t[qs] = hp
                rt_all = kp.tile([128, NQB, D], F16, tag="resid", bufs=1,
                                 name=f"rt{b}")
                nc.sync.dma_start(out=rt_all,
                                  in_=res_d[b].rearrange("a p d -> p a d"))

                # previous batch's LN applies + out-DMAs go AFTER this
                # batch's residual DMA in the SP queue (a blocked out-DMA at
                # the SP head would starve the residual load for ~100us)
                for it in apply_items:
                    it()
                items, applies = fc_items(b, rt_all, hp_t, hidT, te, hz)
                INJECT = False
                if INJECT and b + 1 < BPC:
                    inject_q.extend(items)
                    apply_items = applies
                else:
                    for it in items:
                        it()
                    for it in applies:
                        it()
    nc.compile()
    return nc


_NC_CACHE = None


def _get_module():
    global _NC_CACHE
    if _NC_CACHE is None:
        _NC_CACHE = build_module()
    return _NC_CACHE


def make_in_maps(inputs: dict) -> list[dict]:
    f = lambda x: np.ascontiguousarray(np.asarray(x, dtype=np.float32))
    iq, ik, iv = f(inputs["input_Q"]), f(inputs["input_K"]), f(inputs["input_V"])
    a_ds = f(inputs["A_ds"])
    wq, wk, wv, wfc = f(inputs["Wq"]), f(inputs["Wk"]), f(inputs["Wv"]), f(inputs["Wfc"])
    a1w, a1b = f(inputs["a1_w"]), f(inputs["a1_b"])
    a2w, a2b = f(inputs["a2_w"]), f(inputs["a2_b"])

    h16 = lambda x: np.ascontiguousarray(np.asarray(x, dtype=np.float16))
    e4 = mybir.dt.np(F8)
    h8 = lambda x: np.ascontiguousarray(np.asarray(x, dtype=e4))

    def wpack(w, width=D):
        # [640, width] -> [128, 6, width] with (p, c, m) = w[128c+p, m];
        # chunk 5 is zero padding for the DoubleRow slab pairing
        t = np.zeros((128, NDC + 1, width), dtype=e4)
        t[:, :NDC, :] = h8(w.reshape(NDC, 128, width).transpose(1, 0, 2))
        return t

    w1p = (wfc.astype(np.float64) @ a1w.astype(np.float64)).astype(np.float32)
    shared = {
        "at": h16(np.ascontiguousarray(a_ds.T).reshape(NKB, 128, L)
                  .transpose(1, 0, 2)),
        "w8q": wpack(wq),
        "w8k": wpack(wk),
        "w8v": wpack(wv),
        "wfc8": wpack(wfc),
        "w1p8": wpack(w1p, width=8),
        "a1b": np.ascontiguousarray(a1b.reshape(8, 1), dtype=np.float32),
        "a2s": h16(np.concatenate([a2w, a2b.reshape(1, D)], axis=0)),
    }

    def xpack(x):
        # [BPC, L, D] -> [BPC, 128, 6, L] with (b, p, c, q) = x[b, q, 128c+p]
        t = np.zeros((BPC, 128, NDC + 1, L), dtype=e4)
        t[:, :, :NDC, :] = h8(x.transpose(0, 2, 1).reshape(BPC, NDC, 128, L)
                              .transpose(0, 2, 1, 3))
        return t

    in_maps = []
    for c in range(NCORES):
        sl = slice(c * BPC, (c + 1) * BPC)
        m = dict(shared)
        m["xq8"] = xpack(iq[sl])
        m["xk8"] = xpack(ik[sl])
        m["xv8"] = xpack(iv[sl])
        m["resid"] = h16(iq[sl].reshape(BPC, NQB, 128, D))
        in_maps.append(m)
    return in_maps


_JIT_CACHE = None


def _get_jitted():
    """Build the 8-core shard_map executable once per process.

    run_bass_kernel_spmd re-traces jax on every call (~250ms); caching the
    jitted callable makes repeat kernel() calls cheap."""
    global _JIT_CACHE
    if _JIT_CACHE is not None:
        return _JIT_CACHE
    import jax
    from jax.sharding import Mesh, PartitionSpec
    from jax.experimental.shard_map import shard_map
    from concourse import mybir
    from concourse.bass2jax import (
        _bass_exec_p, install_neuronx_cc_hook, partition_id_tensor)

    nc = _get_module()
    install_neuronx_cc_hook()
    pname = nc.partition_id_tensor.name if nc.partition_id_tensor else None
    in_names, out_names, out_avals, zero_shapes = [], [], [], []
    for alloc in nc.m.functions[0].allocations:
        if not isinstance(alloc, mybir.MemoryLocationSet):
            continue
        name = alloc.memorylocations[0].name
        if alloc.kind == "ExternalInput":
            if name != pname:
                in_names.append(name)
        elif alloc.kind == "ExternalOutput":
            shape = tuple(alloc.tensor_shape)
            dtype = mybir.dt.np(alloc.dtype)
            out_names.append(name)
            out_avals.append(jax.core.ShapedArray(shape, dtype))
            zero_shapes.append((shape, dtype))
    all_in = list(in_names) + list(out_names)
    if pname is not None:
        all_in.append(pname)

    def _body(*args):
        operands = list(args)
        if pname is not None:
            operands.append(partition_id_tensor())
        return tuple(_bass_exec_p.bind(
            *operands, out_avals=tuple(out_avals), in_names=tuple(all_in),
            out_names=tuple(out_names), lowering_input_output_aliases=(),
            sim_require_finite=True, sim_require_nnan=True, nc=nc))

    devices = jax.devices()[:NCORES]
    mesh = Mesh(np.asarray(devices), ("core",))
    n = len(in_names) + len(out_names)
    sharded = jax.jit(
        shard_map(_body, mesh=mesh, in_specs=(PartitionSpec("core"),) * n,
                  out_specs=(PartitionSpec("core"),) * len(out_names),
                  check_rep=False),
        keep_unused=True,
    )
    _JIT_CACHE = (sharded, in_names, zero_shapes)
    return _JIT_CACHE


def kernel(**inputs) -> np.ndarray:
    in_maps = make_in_maps(inputs)
    try:
        sharded, in_names, zero_shapes = _get_jitted()
        concat_in = [
            np.concatenate([np.asarray(in_maps[c][nm]) for c in range(NCORES)],
                           axis=0)
            for nm in in_names
        ]
        concat_zeros = [
            np.zeros((NCORES * s[0], *s[1:]), d) for s, d in zero_shapes
        ]
        outs = sharded(*concat_in, *concat_zeros)
        return np.asarray(outs[0]).astype(np.float32).reshape(B, L, D)
    except Exception:
        nc = _get_module()
        res = run_bass_kernel_spmd(nc, in_maps, core_ids=list(range(NCORES)))
        return np.concatenate([r["out"] for r in res.results],
                              axis=0).astype(np.float32)
